# revision 20
# baseline (speedup 1.0000x reference)
# Bass/Trainium2 kernel for nn_BasicBlock_Sparse (topk sparse-coding basic block).
# Self-contained: builds the Bass program, shards batch across 8 NeuronCores,
# runs via run_bass_kernel_spmd, reassembles full outputs.
#
# Numerics: all matmuls feeding the top-k selections (conv0, conv1, proj0) run
# in fp32. float32r (reduced-precision fast path) is used only where the result
# feeds a tolerant scalar (aux losses) or the final BN'd output (proj1).
import sys
import numpy as np

if "/opt/trn_rl_repo" not in sys.path:
    sys.path.insert(0, "/opt/trn_rl_repo")

N_TOT = 64
NCORES = 8
NIMG = N_TOT // NCORES
C0 = 64
CO = 128
F = 512
K = 16
H = W = 56
OH = OW = 28
PIX = NIMG * OH * OW
BN_EPS = 1e-5

STRIP = 4                  # coarse rows per topk strip -> 112 pixels
NSTRIP = OH // STRIP
M = STRIP * OW             # 112
GR = 14                    # coarse rows per adjoint group -> N=392
NEG = -1e30
TAPS = [(a, b) for a in range(3) for b in range(3)]
# conv0 tap pairing via duplicated shifted image: pairs differ by (+1,+1)
PAIRS = [((0, 0), (1, 1)), ((0, 1), (1, 2)), ((1, 0), (2, 1))]
SINGLES = [(0, 2), (2, 0), (2, 2)]
ADJ_BF16 = True            # adjoints + proj1 in bf16 (aux/output tolerance allows)


def build(n_cores=NCORES, dbg=False):
    nb = n_cores * NIMG
    s0_denom = float(nb * C0 * H * W)
    s1_denom = float(nb * CO * OH * OW)
    bn_denom = float(nb * OH * OW)
    from contextlib import ExitStack
    from concourse import bass, bacc, mybir, tile
    from concourse.masks import make_identity

    F32 = mybir.dt.float32
    F32R = mybir.dt.bfloat16 if ADJ_BF16 else mybir.dt.float32
    AF = mybir.ActivationFunctionType
    OP = mybir.AluOpType

    nc = bacc.Bacc(target_bir_lowering=False, debug=False, num_devices=n_cores)

    x_e = nc.declare_dram_parameter("x", [NIMG, C0, H, W], F32, isOutput=False)
    we0_e = nc.declare_dram_parameter("w_enc0", [F, C0, 3, 3], F32, isOutput=False)
    wp0_e = nc.declare_dram_parameter("w_proj0", [CO, F, 1, 1], F32, isOutput=False)
    g0_e = nc.declare_dram_parameter("g0", [CO], F32, isOutput=False)
    b0_e = nc.declare_dram_parameter("b0", [CO], F32, isOutput=False)
    we1_e = nc.declare_dram_parameter("w_enc1", [F, CO, 3, 3], F32, isOutput=False)
    wp1_e = nc.declare_dram_parameter("w_proj1", [CO, F, 1, 1], F32, isOutput=False)
    g1_e = nc.declare_dram_parameter("g1", [CO], F32, isOutput=False)
    b1_e = nc.declare_dram_parameter("b1", [CO], F32, isOutput=False)
    wsc_e = nc.declare_dram_parameter("w_sc", [CO, C0, 1, 1], F32, isOutput=False)
    gsc_e = nc.declare_dram_parameter("g_sc", [CO], F32, isOutput=False)
    bsc_e = nc.declare_dram_parameter("b_sc", [CO], F32, isOutput=False)

    out_e = nc.declare_dram_parameter("out", [NIMG, CO, OH, OW], F32, isOutput=True)
    aux_e = nc.declare_dram_parameter("aux", [1, 2], F32, isOutput=True)

    dbg_o = {}
    if dbg:
        for nm, shp in [("d_a0", [NSTRIP, M, F]), ("d_sp0T", [128, 4, OH * OW]),
                        ("d_y0T", [CO, PIX]), ("d_ysc", [CO, PIX]),
                        ("d_out0", [CO, NIMG, 30, 30]), ("d_y1T", [CO, PIX]),
                        ("d_recon", [C0, 15, 29])]:
            dbg_o[nm] = nc.declare_dram_parameter(nm, shp, F32, isOutput=True)

    aspace = {"addr_space": "Shared"} if n_cores > 4 else {}
    ar1_in = nc.dram_tensor("ar1_in", [513, 1], F32)
    ar1_out = nc.dram_tensor("ar1_out", [513, 1], F32, **aspace)
    ar2_in = nc.dram_tensor("ar2_in", [257, 1], F32)
    ar2_out = nc.dram_tensor("ar2_out", [257, 1], F32, **aspace)
    groups = [list(range(n_cores))]

    with tile.TileContext(nc) as tc, ExitStack() as ctx:
        P0 = ctx.enter_context(tc.tile_pool(name="P0", bufs=1))
        scr = ctx.enter_context(tc.tile_pool(name="scr", bufs=3))
        a0p = ctx.enter_context(tc.tile_pool(name="a0p", bufs=2))
        spz = ctx.enter_context(tc.tile_pool(name="spz", bufs=3))
        mxp = ctx.enter_context(tc.tile_pool(name="mxp", bufs=4))
        spSp = ctx.enter_context(tc.tile_pool(name="spSp", bufs=2))   # f32 spT strip (proj0 rhs)
        spTp = ctx.enter_context(tc.tile_pool(name="spTp", bufs=2))   # f32r spT per image
        psA = ctx.enter_context(tc.tile_pool(name="psA", bufs=2, space="PSUM"))
        psM = ctx.enter_context(tc.tile_pool(name="psM", bufs=2, space="PSUM"))
        psJ = ctx.enter_context(tc.tile_pool(name="psJ", bufs=2, space="PSUM"))

        ident = P0.tile([128, 128], F32)
        make_identity(nc, ident[:])
        ones128 = P0.tile([128, 1], F32)
        nc.vector.memset(ones128[:], 1.0)

        yscT = P0.tile([CO, PIX], F32)
        xpad1 = P0.tile([CO, NIMG, 30, 30], F32)

        y0_st = P0.tile([CO, 2, NSTRIP * NIMG], F32)
        sc_st = P0.tile([CO, 2, 2 * NIMG], F32)
        y1_st = P0.tile([CO, 2, 2 * NIMG], F32)
        aux0_acc = P0.tile([C0, 4 * 2 * NIMG], F32)
        aux1_acc = P0.tile([CO, 2 * NIMG], F32)
        sv = P0.tile([CO, 16], F32)
        nc.vector.memset(sv[:, 15:16], BN_EPS)
        aux_sb = P0.tile([1, 4], F32)

        def topk_select(a_sb):
            m1 = mxp.tile([M, 8], F32, tag="m8")
            az = spz.tile([M, F], F32, tag="spz")
            m2 = mxp.tile([M, 8], F32, tag="m8")
            sp = spz.tile([M, F], F32, tag="spz")
            nc.vector.max(out=m1[:], in_=a_sb[:])
            nc.vector.match_replace(out=az[:], in_to_replace=m1[:], in_values=a_sb[:], imm_value=NEG)
            nc.vector.max(out=m2[:], in_=az[:])
            nc.vector.scalar_tensor_tensor(
                out=sp[:], in0=a_sb[:], scalar=m2[:, 7:8], in1=a_sb[:],
                op0=OP.is_ge, op1=OP.mult)
            return sp

        def bn_coeffs(s_col, q_col, g_ext, b_ext, so, to):
            mcol, vcol, icol, tmp = sv[:, 8:9], sv[:, 9:10], sv[:, 10:11], sv[:, 11:12]
            gcol, bcol = sv[:, 12:13], sv[:, 13:14]
            nc.sync.dma_start(gcol, g_ext[:].unsqueeze(1))
            nc.sync.dma_start(bcol, b_ext[:].unsqueeze(1))
            nc.scalar.mul(mcol, s_col, 1.0 / bn_denom)
            nc.scalar.activation(out=tmp, in_=mcol, func=AF.Square)
            nc.vector.scalar_tensor_tensor(out=vcol, in0=q_col, scalar=1.0 / bn_denom,
                                           in1=tmp, op0=OP.mult, op1=OP.subtract)
            nc.scalar.activation(out=tmp, in_=vcol, func=AF.Sqrt, bias=sv[:, 15:16])
            nc.vector.reciprocal(icol, tmp)
            nc.vector.tensor_mul(so, gcol, icol)
            nc.vector.tensor_mul(tmp, mcol, so)
            nc.vector.tensor_sub(to, bcol, tmp)

        def cross_part_sum(acc_col, np_, out_slot):
            ps = psM.tile([1, 1], F32, tag="mid")
            nc.tensor.matmul(ps[:], lhsT=ones128[:np_], rhs=acc_col, start=True, stop=True)
            nc.scalar.copy(aux_sb[0:1, out_slot:out_slot + 1], ps[:])

        # =============== preamble: layer-0 weights ===============
        ctx1 = ctx.enter_context(ExitStack())
        P1 = ctx1.enter_context(tc.tile_pool(name="P1", bufs=1))
        # paired conv0 weights: [128, pair, F]; rows 0:64 tap a, 64:128 tap b
        we0_pair = P1.tile([128, 3, F], F32)
        we0_single = P1.tile([C0, 3, F], F32)
        w0r = we0_e[:].rearrange("f c a b -> c (a b) f")   # [64, 9, 512]
        for p, (ta, tb) in enumerate(PAIRS):
            nc.sync.dma_start(we0_pair[0:C0, p], w0r[:, ta[0] * 3 + ta[1]])
            nc.sync.dma_start(we0_pair[C0:128, p], w0r[:, tb[0] * 3 + tb[1]])
        for q, (ki, kj) in enumerate(SINGLES):
            nc.sync.dma_start(we0_single[:, q], w0r[:, ki * 3 + kj])
        # adjoint weights [fp, fb, c, tap] fp32 -> f32r copy
        we0_adj = P1.tile([128, 4, C0, 9], F32)
        for fb in range(4):
            nc.sync.dma_start(we0_adj[:, fb],
                              we0_e[fb * 128:(fb + 1) * 128].rearrange("f c a b -> f c (a b)"))
        we0_adj_r = P1.tile([128, 4, 9, C0], F32R)
        nc.scalar.copy(we0_adj_r[:], we0_adj[:].rearrange("p a c t -> p a t c"))
        # w_proj0 -> [f, cout] fp32 (feeds conv1 input -> keep exact)
        wp0_sb = P1.tile([CO, F], F32)
        nc.sync.dma_start(wp0_sb[:], wp0_e[:].rearrange("o f a b -> o (f a b)"))
        wp0T = P1.tile([128, 4, CO], F32)
        for fb in range(4):
            pt = psM.tile([CO, 512], F32, tag="mid")
            nc.tensor.transpose(pt[:, :128], wp0_sb[:, fb * 128:(fb + 1) * 128], ident[:])
            nc.scalar.copy(wp0T[:, fb], pt[:, :128])
        wsc_sb = P1.tile([CO, C0], F32)
        nc.sync.dma_start(wsc_sb[:], wsc_e[:].rearrange("o c a b -> o (c a b)"))
        wscT = P1.tile([C0, CO], F32)
        ptw = psM.tile([CO, 512], F32, tag="mid")
        nc.tensor.transpose(ptw[:C0, :CO], wsc_sb[:], ident[:])
        nc.scalar.copy(wscT[:], ptw[:C0, :CO])

        y0T = P1.tile([CO, PIX], F32)
        xpool = ctx1.enter_context(tc.tile_pool(name="xpool", bufs=2))

        # =============== phase 1: layer 0 + shortcut ===============
        for i in range(NIMG):
            # xp: partitions 0:64 = padded image A; 64:128 = image at origin B
            # so that A[c, r+1, j+1] == B[c, r, j]  (tap pairs differing by (+1,+1))
            xp = xpool.tile([128, 58, 58], F32, tag="xpad")
            nc.vector.memset(xp[0:C0, 0], 0.0)
            nc.vector.memset(xp[0:C0, 57], 0.0)
            nc.vector.memset(xp[0:C0, 1:57, 0:1], 0.0)
            nc.vector.memset(xp[0:C0, 1:57, 57:58], 0.0)
            nc.vector.memset(xp[C0:128, 56:58], 0.0)
            nc.vector.memset(xp[C0:128, 0:56, 56:58], 0.0)
            nc.sync.dma_start(xp[0:C0, 1:57, 1:57], x_e[i])
            nc.sync.dma_start(xp[C0:128, 0:56, 0:56], x_e[i])

            # ---- shortcut 1x1 stride-2 (fp32, cheap) ----
            for h in range(2):
                ps = psM.tile([CO, 512], F32, tag="mid")
                r0 = 1 + 2 * (h * GR)
                nc.tensor.matmul(
                    ps[:, :392], lhsT=wscT[:],
                    rhs=xp[0:C0, r0:r0 + 2 * GR:2, 1:57:2],
                    start=True, stop=True)
                col = i * 784 + h * 392
                idx = 2 * i + h
                nc.scalar.activation(out=yscT[:, col:col + 392], in_=ps[:, :392],
                                     func=AF.Copy, accum_out=sc_st[:, 0, idx:idx + 1])
                sq = scr.tile([CO, 512], F32, tag="scratch")
                nc.scalar.activation(out=sq[:, :392], in_=ps[:, :392], func=AF.Square,
                                     accum_out=sc_st[:, 1, idx:idx + 1])

            # ---- conv0 strips + topk + transpose + per-strip proj0 ----
            spT_r = spTp.tile([128, 4, OH * OW], F32R, tag="spT")
            for s in range(NSTRIP):
                aT = a0p.tile([128, 4, M], F32, tag="aT")
                nmm = len(PAIRS) + len(SINGLES)
                for fb in range(4):
                    fsl = slice(fb * 128, (fb + 1) * 128)
                    paT = psA.tile([128, M], F32, tag="aT_ps")
                    t = 0
                    for p, ((ki, kj), _) in enumerate(PAIRS):
                        nc.tensor.matmul(
                            paT[:], lhsT=we0_pair[:, p, fsl],
                            rhs=xp[:, 8 * s + ki:8 * s + ki + 8:2, kj:kj + 56:2],
                            start=(t == 0), stop=(t == nmm - 1))
                        t += 1
                    for q, (ki, kj) in enumerate(SINGLES):
                        nc.tensor.matmul(
                            paT[:], lhsT=we0_single[:, q, fsl],
                            rhs=xp[0:C0, 8 * s + ki:8 * s + ki + 8:2, kj:kj + 56:2],
                            start=(t == 0), stop=(t == nmm - 1))
                        t += 1
                    nc.scalar.copy(aT[:, fb], paT[:])
                pa = psA.tile([M, F], F32, tag="a")
                for fb in range(4):
                    nc.tensor.transpose(pa[:, fb * 128:(fb + 1) * 128], aT[:, fb], ident[:])
                a_sb = a0p.tile([M, F], F32, tag="a_sb")
                nc.scalar.copy(a_sb[:], pa[:])
                if dbg and i == 0:
                    nc.sync.dma_start(dbg_o["d_a0"][s], a_sb[:])
                sp = topk_select(a_sb)
                spS = spSp.tile([128, 4, M], F32, tag="spS")
                for fb in range(4):
                    pt = psM.tile([CO, 512], F32, tag="mid")
                    nc.tensor.transpose(pt[:, :M], sp[:, fb * 128:(fb + 1) * 128],
                                        ident[:M, :M])
                    nc.scalar.copy(spS[:, fb], pt[:, :M])
                    nc.scalar.copy(spT_r[:, fb, s * M:(s + 1) * M], pt[:, :M])
                # proj0 per strip (fp32: feeds conv1 input)
                py = psM.tile([CO, 512], F32, tag="mid")
                for fb in range(4):
                    nc.tensor.matmul(py[:, :M], lhsT=wp0T[:, fb], rhs=spS[:, fb],
                                     start=(fb == 0), stop=(fb == 3))
                sidx = NSTRIP * i + s
                nc.scalar.activation(out=y0T[:, sidx * M:(sidx + 1) * M], in_=py[:, :M],
                                     func=AF.Copy, accum_out=y0_st[:, 0, sidx:sidx + 1])
                sq = scr.tile([CO, 512], F32, tag="scratch")
                nc.scalar.activation(out=sq[:, :M], in_=py[:, :M], func=AF.Square,
                                     accum_out=y0_st[:, 1, sidx:sidx + 1])
            if dbg and i == 0:
                for fb in range(4):
                    dcp = scr.tile([CO, 512], F32, tag="scratch")
                    nc.scalar.copy(dcp[:, :392], spT_r[:, fb, 0:392])
                    nc.sync.dma_start(dbg_o["d_sp0T"][:, fb, 0:392], dcp[:, :392])
                    dcp2 = scr.tile([CO, 512], F32, tag="scratch")
                    nc.scalar.copy(dcp2[:, :392], spT_r[:, fb, 392:784])
                    nc.sync.dma_start(dbg_o["d_sp0T"][:, fb, 392:784], dcp2[:, :392])

            # ---- adjoint0 (f32r) by output polyphase ----
            for ip, (pi, pj) in enumerate([(0, 0), (0, 1), (1, 0), (1, 1)]):
                ptaps = [(t_, (ki - pi) // 2, (kj - pj) // 2)
                         for t_, (ki, kj) in enumerate(TAPS)
                         if ki % 2 == pi and kj % 2 == pj]
                nr = 15 if pi == 0 else 14
                ncl = 29 if pj == 0 else 28
                prev = None
                for g in range(2):
                    pr = psJ.tile([C0, 15, 29], F32, tag="adj")
                    nc.vector.memset(pr[:, :nr, :ncl], 0.0)
                    for t_, dr, dc in ptaps:
                        for fb in range(4):
                            nc.tensor.matmul(
                                pr[:, dr:dr + GR, dc:dc + OW],
                                lhsT=we0_adj_r[:, fb, t_],
                                rhs=spT_r[:, fb, g * 392:(g + 1) * 392],
                                start=False, stop=False, skip_group_check=True)
                    if dbg and i == 0 and ip == 0 and g == 0:
                        drb = scr.tile([CO, 512], F32, tag="scratch")
                        nc.scalar.copy(drb[:C0, :435], pr[:].rearrange("p a b -> p (a b)"))
                        nc.sync.dma_start(dbg_o["d_recon"][:].rearrange("p a b -> p (a b)"), drb[:C0, :435])
                    if pi == 0:
                        if g == 0:
                            rows = (1, 14)
                        else:
                            cb = scr.tile([CO, 512], F32, tag="scratch")
                            nc.scalar.copy(cb[:C0, :ncl], prev[:, 14, :ncl])
                            nc.vector.tensor_add(pr[:, 0, :ncl], pr[:, 0, :ncl],
                                                 cb[:C0, :ncl])
                            rows = (0, 15)
                    else:
                        rows = (0, 14)
                    c0_, c1_ = (1, ncl) if pj == 0 else (0, ncl)
                    nrp = rows[1] - rows[0]
                    ncp = c1_ - c0_
                    ar0 = g * GR + rows[0]
                    xr0 = 2 * ar0 + pi
                    xc0 = 2 * c0_ + pj
                    d = scr.tile([CO, 512], F32, tag="scratch")
                    dv = d[:C0, :nrp * ncp].rearrange("p (a b) -> p a b", a=nrp)
                    nc.vector.scalar_tensor_tensor(
                        out=dv, in0=pr[:, rows[0]:rows[1], c0_:c1_], scalar=1.0,
                        in1=xp[0:C0, xr0:xr0 + 2 * nrp:2, xc0:xc0 + 2 * ncp:2],
                        op0=OP.mult, op1=OP.subtract)
                    d2 = scr.tile([CO, 512], F32, tag="scratch")
                    nc.scalar.activation(out=d2[:C0, :nrp * ncp], in_=d[:C0, :nrp * ncp],
                                         func=AF.Square,
                                         accum_out=aux0_acc[:, 8 * i + 2 * ip + g:8 * i + 2 * ip + g + 1])
                    prev = pr

        # =============== allreduce 1 ===============
        nc.vector.reduce_sum(sv[:, 0:1], y0_st[:, 0], axis=mybir.AxisListType.X)
        nc.vector.reduce_sum(sv[:, 1:2], y0_st[:, 1], axis=mybir.AxisListType.X)
        nc.vector.reduce_sum(sv[:, 2:3], sc_st[:, 0], axis=mybir.AxisListType.X)
        nc.vector.reduce_sum(sv[:, 3:4], sc_st[:, 1], axis=mybir.AxisListType.X)
        a0c = P0.tile([C0, 1], F32)
        nc.vector.reduce_sum(a0c[:], aux0_acc[:], axis=mybir.AxisListType.X)
        cross_part_sum(a0c[:], C0, 0)
        nc.sync.dma_start(ar1_in[0:128], sv[:, 0:1])
        nc.sync.dma_start(ar1_in[128:256], sv[:, 1:2])
        nc.sync.dma_start(ar1_in[256:384], sv[:, 2:3])
        nc.sync.dma_start(ar1_in[384:512], sv[:, 3:4])
        nc.sync.dma_start(ar1_in[512:513], aux_sb[0:1, 0:1])
        nc.gpsimd.collective_compute(
            "AllReduce", mybir.AluOpType.add, replica_groups=groups,
            ins=[ar1_in[:]], outs=[ar1_out[:]])
        nc.sync.dma_start(sv[:, 0:1], ar1_out[0:128])
        nc.sync.dma_start(sv[:, 1:2], ar1_out[128:256])
        nc.sync.dma_start(sv[:, 2:3], ar1_out[256:384])
        nc.sync.dma_start(sv[:, 3:4], ar1_out[384:512])
        nc.sync.dma_start(aux_sb[0:1, 0:1], ar1_out[512:513])

        s0c, t0c = sv[:, 4:5], sv[:, 5:6]
        bn_coeffs(sv[:, 0:1], sv[:, 1:2], g0_e, b0_e, s0c, t0c)
        ssc, tsc = sv[:, 6:7], sv[:, 7:8]
        bn_coeffs(sv[:, 2:3], sv[:, 3:4], gsc_e, bsc_e, ssc, tsc)
        nc.scalar.mul(aux_sb[0:1, 2:3], aux_sb[0:1, 0:1], 1.0 / s0_denom)

        # =============== build out0 (xpad1) ===============
        nc.vector.memset(xpad1[:, :, 0], 0.0)
        nc.vector.memset(xpad1[:, :, 29], 0.0)
        nc.vector.memset(xpad1[:, :, 1:29, 0:1], 0.0)
        nc.vector.memset(xpad1[:, :, 1:29, 29:30], 0.0)
        nc.scalar.activation(
            out=xpad1[:, :, 1:29, 1:29],
            in_=y0T[:].rearrange("p (n h w) -> p n h w", n=NIMG, h=OH),
            func=AF.Identity, scale=s0c, bias=t0c)
        if dbg:
            nc.sync.dma_start(dbg_o["d_y0T"][:], y0T[:])
            nc.sync.dma_start(dbg_o["d_ysc"][:], yscT[:])
            nc.sync.dma_start(dbg_o["d_out0"][:], xpad1[:])

        ctx1.close()   # free layer-0 weights, y0T, xpool

        # =============== phase 2: layer 1 ===============
        P2 = ctx.enter_context(tc.tile_pool(name="P2", bufs=1))
        we1_rhs = P2.tile([CO, 9, F], F32)
        nc.sync.dma_start(we1_rhs[:], we1_e[:].rearrange("f c a b -> c (a b) f"))
        we1_adj = P2.tile([128, 4, CO, 9], F32)
        for fb in range(4):
            nc.sync.dma_start(we1_adj[:, fb],
                              we1_e[fb * 128:(fb + 1) * 128].rearrange("f c a b -> f c (a b)"))
        we1_adj_r = P2.tile([128, 4, 9, CO], F32R)
        nc.scalar.copy(we1_adj_r[:], we1_adj[:].rearrange("p a c t -> p a t c"))
        wp1_sb = P2.tile([CO, F], F32)
        nc.sync.dma_start(wp1_sb[:], wp1_e[:].rearrange("o f a b -> o (f a b)"))
        wp1T_r = P2.tile([128, 4, CO], F32R)
        for fb in range(4):
            pt = psM.tile([CO, 512], F32, tag="mid")
            nc.tensor.transpose(pt[:, :128], wp1_sb[:, fb * 128:(fb + 1) * 128], ident[:])
            nc.scalar.copy(wp1T_r[:, fb], pt[:, :128])
        y1T = P2.tile([CO, PIX], F32)

        for i in range(NIMG):
            spT_r = spTp.tile([128, 4, OH * OW], F32R, tag="spT")
            for s in range(NSTRIP):
                aT = a0p.tile([128, 4, M], F32, tag="aT")
                for fb in range(4):
                    fsl = slice(fb * 128, (fb + 1) * 128)
                    paT = psA.tile([128, M], F32, tag="aT_ps")
                    for t, (ki, kj) in enumerate(TAPS):
                        nc.tensor.matmul(
                            paT[:], lhsT=we1_rhs[:, t, fsl],
                            rhs=xpad1[:, i, 4 * s + ki:4 * s + ki + 4, kj:kj + 28],
                            start=(t == 0), stop=(t == 8))
                    nc.scalar.copy(aT[:, fb], paT[:])
                pa = psA.tile([M, F], F32, tag="a")
                for fb in range(4):
                    nc.tensor.transpose(pa[:, fb * 128:(fb + 1) * 128], aT[:, fb], ident[:])
                a_sb = a0p.tile([M, F], F32, tag="a_sb")
                nc.scalar.copy(a_sb[:], pa[:])
                sp = topk_select(a_sb)
                for fb in range(4):
                    pt = psM.tile([CO, 512], F32, tag="mid")
                    nc.tensor.transpose(pt[:, :M], sp[:, fb * 128:(fb + 1) * 128],
                                        ident[:M, :M])
                    nc.scalar.copy(spT_r[:, fb, s * M:(s + 1) * M], pt[:, :M])

            # proj1 (f32r: output-only) per half-image
            for h in range(2):
                py = psM.tile([CO, 512], F32, tag="mid")
                for fb in range(4):
                    nc.tensor.matmul(
                        py[:, :392], lhsT=wp1T_r[:, fb],
                        rhs=spT_r[:, fb, h * 392:(h + 1) * 392],
                        start=(fb == 0), stop=(fb == 3))
                col = i * 784 + h * 392
                idx = 2 * i + h
                nc.scalar.activation(out=y1T[:, col:col + 392], in_=py[:, :392],
                                     func=AF.Copy, accum_out=y1_st[:, 0, idx:idx + 1])
                sq = scr.tile([CO, 512], F32, tag="scratch")
                nc.scalar.activation(out=sq[:, :392], in_=py[:, :392], func=AF.Square,
                                     accum_out=y1_st[:, 1, idx:idx + 1])

            # adjoint1 (f32r)
            prev = None
            for g in range(2):
                pr = psJ.tile([CO, 16, 30], F32, tag="adj")
                nc.vector.memset(pr[:], 0.0)
                for t, (ki, kj) in enumerate(TAPS):
                    for fb in range(4):
                        nc.tensor.matmul(
                            pr[:, ki:ki + GR, kj:kj + OW],
                            lhsT=we1_adj_r[:, fb, t],
                            rhs=spT_r[:, fb, g * 392:(g + 1) * 392],
                            start=False, stop=False, skip_group_check=True)
                if g == 0:
                    rows = (1, 14)
                else:
                    cb = scr.tile([CO, 512], F32, tag="scratch")
                    nc.scalar.copy(cb[:, :60], prev[:, 14:16].rearrange("p a b -> p (a b)"))
                    nc.vector.tensor_add(pr[:, 0:2], pr[:, 0:2],
                                         cb[:, :60].rearrange("p (a b) -> p a b", a=2))
                    rows = (0, 15)
                nrp = rows[1] - rows[0]
                ar0 = g * GR + rows[0]
                d = scr.tile([CO, 512], F32, tag="scratch")
                dv = d[:, :nrp * 28].rearrange("p (a b) -> p a b", a=nrp)
                nc.vector.scalar_tensor_tensor(
                    out=dv, in0=pr[:, rows[0]:rows[1], 1:29], scalar=1.0,
                    in1=xpad1[:, i, ar0:ar0 + nrp, 1:29],
                    op0=OP.mult, op1=OP.subtract)
                d2 = scr.tile([CO, 512], F32, tag="scratch")
                nc.scalar.activation(out=d2[:, :nrp * 28], in_=d[:, :nrp * 28],
                                     func=AF.Square,
                                     accum_out=aux1_acc[:, 2 * i + g:2 * i + g + 1])
                prev = pr

        # =============== allreduce 2 ===============
        nc.vector.reduce_sum(sv[:, 0:1], y1_st[:, 0], axis=mybir.AxisListType.X)
        nc.vector.reduce_sum(sv[:, 1:2], y1_st[:, 1], axis=mybir.AxisListType.X)
        a1c = P0.tile([CO, 1], F32)
        nc.vector.reduce_sum(a1c[:], aux1_acc[:], axis=mybir.AxisListType.X)
        cross_part_sum(a1c[:], CO, 1)
        nc.sync.dma_start(ar2_in[0:128], sv[:, 0:1])
        nc.sync.dma_start(ar2_in[128:256], sv[:, 1:2])
        nc.sync.dma_start(ar2_in[256:257], aux_sb[0:1, 1:2])
        nc.gpsimd.collective_compute(
            "AllReduce", mybir.AluOpType.add, replica_groups=groups,
            ins=[ar2_in[:]], outs=[ar2_out[:]])
        nc.sync.dma_start(sv[:, 0:1], ar2_out[0:128])
        nc.sync.dma_start(sv[:, 1:2], ar2_out[128:256])
        nc.sync.dma_start(aux_sb[0:1, 1:2], ar2_out[256:257])
        if dbg:
            nc.sync.dma_start(dbg_o["d_y1T"][:], y1T[:])

        s1c, t1c = sv[:, 4:5], sv[:, 5:6]
        bn_coeffs(sv[:, 0:1], sv[:, 1:2], g1_e, b1_e, s1c, t1c)
        nc.scalar.mul(aux_sb[0:1, 3:4], aux_sb[0:1, 1:2], 1.0 / s1_denom)
        tCc = sv[:, 14:15]
        nc.vector.tensor_add(tCc, t1c, tsc)

        # =============== phase 3: out = relu(s1*y1 + ssc*ysc + tC) ===============
        for c in range(PIX // 392):
            i, h = c // 2, c % 2
            sl = slice(392 * c, 392 * (c + 1))
            tmp = scr.tile([CO, 512], F32, tag="scratch")
            nc.scalar.activation(out=tmp[:, :392], in_=yscT[:, sl],
                                 func=AF.Identity, scale=ssc, bias=tCc)
            tmp2 = scr.tile([CO, 512], F32, tag="scratch")
            nc.vector.scalar_tensor_tensor(
                out=tmp2[:, :392], in0=y1T[:, sl], scalar=s1c, in1=tmp[:, :392],
                op0=OP.mult, op1=OP.add)
            tmp3 = scr.tile([CO, 512], F32, tag="scratch")
            nc.scalar.activation(out=tmp3[:, :392], in_=tmp2[:, :392], func=AF.Relu)
            nc.sync.dma_start(
                out_e[i].rearrange("o h w -> o (h w)")[:, 392 * h:392 * (h + 1)],
                tmp3[:, :392])
        nc.sync.dma_start(aux_e[:], aux_sb[0:1, 2:4])

    nc.finalize()
    return nc


_CACHE = {}


def _get_program(n_cores=NCORES, dbg=False):
    key = (n_cores, dbg)
    if key not in _CACHE:
        _CACHE[key] = build(n_cores, dbg)
    return _CACHE[key]


def kernel(**inputs):
    from concourse.bass_utils import run_bass_kernel_spmd
    nc = _get_program()
    x = np.ascontiguousarray(np.asarray(inputs["x"], dtype=np.float32))
    weights = {k: np.ascontiguousarray(np.asarray(v, dtype=np.float32))
               for k, v in inputs.items() if k != "x"}
    in_maps = []
    for c in range(NCORES):
        m = {"x": x[c * NIMG:(c + 1) * NIMG]}
        m.update(weights)
        in_maps.append(m)
    res = run_bass_kernel_spmd(nc, in_maps, list(range(NCORES)))
    out = np.concatenate([res.results[c]["out"].reshape(NIMG, CO, OH, OW)
                          for c in range(NCORES)], axis=0)
    aux = res.results[0]["aux"].reshape(-1)
    return out, np.float32(aux[0]), np.float32(aux[1])


# revision 22
# speedup vs baseline: 37.2009x; 37.2009x over previous
# Bass/Trainium2 kernel for nn_BasicBlock_Sparse (topk sparse-coding basic block).
# Self-contained: builds the Bass program, shards batch across 8 NeuronCores,
# runs via run_bass_kernel_spmd, reassembles full outputs.
#
# Numerics: all matmuls feeding the top-k selections (conv0, conv1, proj0) run
# in fp32. float32r (reduced-precision fast path) is used only where the result
# feeds a tolerant scalar (aux losses) or the final BN'd output (proj1).
import sys
import numpy as np

if "/opt/trn_rl_repo" not in sys.path:
    sys.path.insert(0, "/opt/trn_rl_repo")

N_TOT = 64
NCORES = 8
NIMG = N_TOT // NCORES
C0 = 64
CO = 128
F = 512
K = 16
H = W = 56
OH = OW = 28
PIX = NIMG * OH * OW
BN_EPS = 1e-5

STRIP = 4                  # coarse rows per topk strip -> 112 pixels
NSTRIP = OH // STRIP
M = STRIP * OW             # 112
GR = 14                    # coarse rows per adjoint group -> N=392
NEG = -1e30
TAPS = [(a, b) for a in range(3) for b in range(3)]
# conv0 tap pairing via duplicated shifted image: pairs differ by (+1,+1)
PAIRS = [((0, 0), (1, 1)), ((0, 1), (1, 2)), ((1, 0), (2, 1))]
SINGLES = [(0, 2), (2, 0), (2, 2)]
ADJ_BF16 = True            # adjoints + proj1 in bf16 (aux/output tolerance allows)


def build(n_cores=NCORES, dbg=False):
    nb = n_cores * NIMG
    s0_denom = float(nb * C0 * H * W)
    s1_denom = float(nb * CO * OH * OW)
    bn_denom = float(nb * OH * OW)
    from contextlib import ExitStack
    from concourse import bass, bacc, mybir, tile
    from concourse.masks import make_identity

    F32 = mybir.dt.float32
    F32R = mybir.dt.bfloat16 if ADJ_BF16 else mybir.dt.float32
    AF = mybir.ActivationFunctionType
    OP = mybir.AluOpType

    nc = bacc.Bacc(target_bir_lowering=False, debug=False, num_devices=n_cores)

    x_e = nc.declare_dram_parameter("x", [NIMG, C0, H, W], F32, isOutput=False)
    we0_e = nc.declare_dram_parameter("w_enc0", [F, C0, 3, 3], F32, isOutput=False)
    wp0_e = nc.declare_dram_parameter("w_proj0", [CO, F, 1, 1], F32, isOutput=False)
    g0_e = nc.declare_dram_parameter("g0", [CO], F32, isOutput=False)
    b0_e = nc.declare_dram_parameter("b0", [CO], F32, isOutput=False)
    we1_e = nc.declare_dram_parameter("w_enc1", [F, CO, 3, 3], F32, isOutput=False)
    wp1_e = nc.declare_dram_parameter("w_proj1", [CO, F, 1, 1], F32, isOutput=False)
    g1_e = nc.declare_dram_parameter("g1", [CO], F32, isOutput=False)
    b1_e = nc.declare_dram_parameter("b1", [CO], F32, isOutput=False)
    wsc_e = nc.declare_dram_parameter("w_sc", [CO, C0, 1, 1], F32, isOutput=False)
    gsc_e = nc.declare_dram_parameter("g_sc", [CO], F32, isOutput=False)
    bsc_e = nc.declare_dram_parameter("b_sc", [CO], F32, isOutput=False)

    out_e = nc.declare_dram_parameter("out", [NIMG, CO, OH, OW], F32, isOutput=True)
    aux_e = nc.declare_dram_parameter("aux", [1, 2], F32, isOutput=True)

    dbg_o = {}
    if dbg:
        for nm, shp in [("d_a0", [NSTRIP, M, F]), ("d_sp0T", [128, 4, OH * OW]),
                        ("d_y0T", [CO, PIX]), ("d_ysc", [CO, PIX]),
                        ("d_out0", [CO, NIMG, 30, 30]), ("d_y1T", [CO, PIX]),
                        ("d_recon", [C0, 15, 29])]:
            dbg_o[nm] = nc.declare_dram_parameter(nm, shp, F32, isOutput=True)

    aspace = {"addr_space": "Shared"} if n_cores > 4 else {}
    ar1_in = nc.dram_tensor("ar1_in", [513, 1], F32)
    ar1_out = nc.dram_tensor("ar1_out", [513, 1], F32, **aspace)
    ar2_in = nc.dram_tensor("ar2_in", [257, 1], F32)
    ar2_out = nc.dram_tensor("ar2_out", [257, 1], F32, **aspace)
    groups = [list(range(n_cores))]

    with tile.TileContext(nc) as tc, ExitStack() as ctx:
        P0 = ctx.enter_context(tc.tile_pool(name="P0", bufs=1))
        scr = ctx.enter_context(tc.tile_pool(name="scr", bufs=3))
        a0p = ctx.enter_context(tc.tile_pool(name="a0p", bufs=2))
        spz = ctx.enter_context(tc.tile_pool(name="spz", bufs=3))
        mxp = ctx.enter_context(tc.tile_pool(name="mxp", bufs=4))
        spSp = ctx.enter_context(tc.tile_pool(name="spSp", bufs=2))   # f32 spT strip (proj0 rhs)
        spTp = ctx.enter_context(tc.tile_pool(name="spTp", bufs=2))   # f32r spT per image
        psA = ctx.enter_context(tc.tile_pool(name="psA", bufs=2, space="PSUM"))
        psM = ctx.enter_context(tc.tile_pool(name="psM", bufs=2, space="PSUM"))
        psJ = ctx.enter_context(tc.tile_pool(name="psJ", bufs=2, space="PSUM"))

        ident = P0.tile([128, 128], F32)
        make_identity(nc, ident[:])
        ones128 = P0.tile([128, 1], F32)
        nc.vector.memset(ones128[:], 1.0)

        yscT = P0.tile([CO, PIX], F32)
        xpad1 = P0.tile([CO, NIMG, 30, 30], F32)

        y0_st = P0.tile([CO, 2, NSTRIP * NIMG], F32)
        sc_st = P0.tile([CO, 2, 2 * NIMG], F32)
        y1_st = P0.tile([CO, 2, 2 * NIMG], F32)
        aux0_acc = P0.tile([C0, 4 * 2 * NIMG], F32)
        aux1_acc = P0.tile([CO, 2 * NIMG], F32)
        sv = P0.tile([CO, 16], F32)
        nc.vector.memset(sv[:, 15:16], BN_EPS)
        aux_sb = P0.tile([1, 4], F32)

        def topk_select(a_sb):
            m1 = mxp.tile([M, 8], F32, tag="m8")
            az = spz.tile([M, F], F32, tag="spz")
            m2 = mxp.tile([M, 8], F32, tag="m8")
            sp = spz.tile([M, F], F32, tag="spz")
            nc.vector.max(out=m1[:], in_=a_sb[:])
            nc.vector.match_replace(out=az[:], in_to_replace=m1[:], in_values=a_sb[:], imm_value=NEG)
            nc.vector.max(out=m2[:], in_=az[:])
            nc.vector.scalar_tensor_tensor(
                out=sp[:], in0=a_sb[:], scalar=m2[:, 7:8], in1=a_sb[:],
                op0=OP.is_ge, op1=OP.mult)
            return sp

        def bn_coeffs(s_col, q_col, g_ext, b_ext, so, to):
            mcol, vcol, icol, tmp = sv[:, 8:9], sv[:, 9:10], sv[:, 10:11], sv[:, 11:12]
            gcol, bcol = sv[:, 12:13], sv[:, 13:14]
            nc.sync.dma_start(gcol, g_ext[:].unsqueeze(1))
            nc.sync.dma_start(bcol, b_ext[:].unsqueeze(1))
            nc.scalar.mul(mcol, s_col, 1.0 / bn_denom)
            nc.scalar.activation(out=tmp, in_=mcol, func=AF.Square)
            nc.vector.scalar_tensor_tensor(out=vcol, in0=q_col, scalar=1.0 / bn_denom,
                                           in1=tmp, op0=OP.mult, op1=OP.subtract)
            nc.scalar.activation(out=tmp, in_=vcol, func=AF.Sqrt, bias=sv[:, 15:16])
            nc.vector.reciprocal(icol, tmp)
            nc.vector.tensor_mul(so, gcol, icol)
            nc.vector.tensor_mul(tmp, mcol, so)
            nc.vector.tensor_sub(to, bcol, tmp)

        def cross_part_sum(acc_col, np_, out_slot):
            ps = psM.tile([1, 1], F32, tag="mid")
            nc.tensor.matmul(ps[:], lhsT=ones128[:np_], rhs=acc_col, start=True, stop=True)
            nc.scalar.copy(aux_sb[0:1, out_slot:out_slot + 1], ps[:])

        # =============== preamble: layer-0 weights ===============
        ctx1 = ctx.enter_context(ExitStack())
        P1 = ctx1.enter_context(tc.tile_pool(name="P1", bufs=1))
        # paired conv0 weights: [128, pair, F]; rows 0:64 tap a, 64:128 tap b
        we0_pair = P1.tile([128, 3, F], F32)
        we0_single = P1.tile([C0, 3, F], F32)
        w0r = we0_e[:].rearrange("f c a b -> c (a b) f")   # [64, 9, 512]
        for p, (ta, tb) in enumerate(PAIRS):
            nc.sync.dma_start(we0_pair[0:C0, p], w0r[:, ta[0] * 3 + ta[1]])
            nc.sync.dma_start(we0_pair[C0:128, p], w0r[:, tb[0] * 3 + tb[1]])
        for q, (ki, kj) in enumerate(SINGLES):
            nc.sync.dma_start(we0_single[:, q], w0r[:, ki * 3 + kj])
        # adjoint weights [fp, fb, c, tap] fp32 -> f32r copy
        we0_adj = P1.tile([128, 4, C0, 9], F32)
        for fb in range(4):
            nc.sync.dma_start(we0_adj[:, fb],
                              we0_e[fb * 128:(fb + 1) * 128].rearrange("f c a b -> f c (a b)"))
        we0_adj_r = P1.tile([128, 4, 9, C0], F32R)
        nc.scalar.copy(we0_adj_r[:], we0_adj[:].rearrange("p a c t -> p a t c"))
        # w_proj0 -> [f, cout] fp32 (feeds conv1 input -> keep exact)
        wp0_sb = P1.tile([CO, F], F32)
        nc.sync.dma_start(wp0_sb[:], wp0_e[:].rearrange("o f a b -> o (f a b)"))
        wp0T = P1.tile([128, 4, CO], F32)
        for fb in range(4):
            pt = psM.tile([CO, 512], F32, tag="mid")
            nc.tensor.transpose(pt[:, :128], wp0_sb[:, fb * 128:(fb + 1) * 128], ident[:])
            nc.scalar.copy(wp0T[:, fb], pt[:, :128])
        wsc_sb = P1.tile([CO, C0], F32)
        nc.sync.dma_start(wsc_sb[:], wsc_e[:].rearrange("o c a b -> o (c a b)"))
        wscT = P1.tile([C0, CO], F32)
        ptw = psM.tile([CO, 512], F32, tag="mid")
        nc.tensor.transpose(ptw[:C0, :CO], wsc_sb[:], ident[:])
        nc.scalar.copy(wscT[:], ptw[:C0, :CO])

        y0T = P1.tile([CO, PIX], F32)
        xpool = ctx1.enter_context(tc.tile_pool(name="xpool", bufs=2))

        # =============== phase 1: layer 0 + shortcut ===============
        for i in range(NIMG):
            # xp: partitions 0:64 = padded image A; 64:128 = image at origin B
            # so that A[c, r+1, j+1] == B[c, r, j]  (tap pairs differing by (+1,+1))
            xp = xpool.tile([128, 58, 58], F32, tag="xpad")
            nc.vector.memset(xp[0:C0, 0], 0.0)
            nc.vector.memset(xp[0:C0, 57], 0.0)
            nc.vector.memset(xp[0:C0, 1:57, 0:1], 0.0)
            nc.vector.memset(xp[0:C0, 1:57, 57:58], 0.0)
            nc.vector.memset(xp[C0:128, 56:58], 0.0)
            nc.vector.memset(xp[C0:128, 0:56, 56:58], 0.0)
            nc.sync.dma_start(xp[0:C0, 1:57, 1:57], x_e[i])
            nc.sync.dma_start(xp[C0:128, 0:56, 0:56], x_e[i])

            # ---- shortcut 1x1 stride-2 (fp32, cheap) ----
            for h in range(2):
                ps = psM.tile([CO, 512], F32, tag="mid")
                r0 = 1 + 2 * (h * GR)
                nc.tensor.matmul(
                    ps[:, :392], lhsT=wscT[:],
                    rhs=xp[0:C0, r0:r0 + 2 * GR:2, 1:57:2],
                    start=True, stop=True)
                col = i * 784 + h * 392
                idx = 2 * i + h
                nc.scalar.activation(out=yscT[:, col:col + 392], in_=ps[:, :392],
                                     func=AF.Copy, accum_out=sc_st[:, 0, idx:idx + 1])
                sq = scr.tile([CO, 512], F32, tag="scratch")
                nc.scalar.activation(out=sq[:, :392], in_=ps[:, :392], func=AF.Square,
                                     accum_out=sc_st[:, 1, idx:idx + 1])

            # ---- conv0 strips + topk + transpose + per-strip proj0 ----
            spT_r = spTp.tile([128, 4, OH * OW], F32R, tag="spT")
            for s in range(NSTRIP):
                aT = a0p.tile([128, 4, M], F32, tag="aT")
                nmm = len(PAIRS) + len(SINGLES)
                for fb in range(4):
                    fsl = slice(fb * 128, (fb + 1) * 128)
                    paT = psA.tile([128, M], F32, tag="aT_ps")
                    t = 0
                    for p, ((ki, kj), _) in enumerate(PAIRS):
                        nc.tensor.matmul(
                            paT[:], lhsT=we0_pair[:, p, fsl],
                            rhs=xp[:, 8 * s + ki:8 * s + ki + 8:2, kj:kj + 56:2],
                            start=(t == 0), stop=(t == nmm - 1))
                        t += 1
                    for q, (ki, kj) in enumerate(SINGLES):
                        nc.tensor.matmul(
                            paT[:], lhsT=we0_single[:, q, fsl],
                            rhs=xp[0:C0, 8 * s + ki:8 * s + ki + 8:2, kj:kj + 56:2],
                            start=(t == 0), stop=(t == nmm - 1))
                        t += 1
                    nc.scalar.copy(aT[:, fb], paT[:])
                pa = psA.tile([M, F], F32, tag="a")
                for fb in range(4):
                    nc.tensor.transpose(pa[:, fb * 128:(fb + 1) * 128], aT[:, fb], ident[:])
                a_sb = a0p.tile([M, F], F32, tag="a_sb")
                nc.scalar.copy(a_sb[:], pa[:])
                if dbg and i == 0:
                    nc.sync.dma_start(dbg_o["d_a0"][s], a_sb[:])
                sp = topk_select(a_sb)
                spS = spSp.tile([128, 4, M], F32, tag="spS")
                for fb in range(4):
                    pt = psM.tile([CO, 512], F32, tag="mid")
                    nc.tensor.transpose(pt[:, :M], sp[:, fb * 128:(fb + 1) * 128],
                                        ident[:M, :M])
                    nc.scalar.copy(spS[:, fb], pt[:, :M])
                    nc.scalar.copy(spT_r[:, fb, s * M:(s + 1) * M], pt[:, :M])
                # proj0 per strip (fp32: feeds conv1 input)
                py = psM.tile([CO, 512], F32, tag="mid")
                for fb in range(4):
                    nc.tensor.matmul(py[:, :M], lhsT=wp0T[:, fb], rhs=spS[:, fb],
                                     start=(fb == 0), stop=(fb == 3))
                sidx = NSTRIP * i + s
                nc.scalar.activation(out=y0T[:, sidx * M:(sidx + 1) * M], in_=py[:, :M],
                                     func=AF.Copy, accum_out=y0_st[:, 0, sidx:sidx + 1])
                sq = scr.tile([CO, 512], F32, tag="scratch")
                nc.scalar.activation(out=sq[:, :M], in_=py[:, :M], func=AF.Square,
                                     accum_out=y0_st[:, 1, sidx:sidx + 1])
            if dbg and i == 0:
                for fb in range(4):
                    dcp = scr.tile([CO, 512], F32, tag="scratch")
                    nc.scalar.copy(dcp[:, :392], spT_r[:, fb, 0:392])
                    nc.sync.dma_start(dbg_o["d_sp0T"][:, fb, 0:392], dcp[:, :392])
                    dcp2 = scr.tile([CO, 512], F32, tag="scratch")
                    nc.scalar.copy(dcp2[:, :392], spT_r[:, fb, 392:784])
                    nc.sync.dma_start(dbg_o["d_sp0T"][:, fb, 392:784], dcp2[:, :392])

            # ---- adjoint0 (f32r) by output polyphase ----
            for ip, (pi, pj) in enumerate([(0, 0), (0, 1), (1, 0), (1, 1)]):
                ptaps = [(t_, (ki - pi) // 2, (kj - pj) // 2)
                         for t_, (ki, kj) in enumerate(TAPS)
                         if ki % 2 == pi and kj % 2 == pj]
                nr = 15 if pi == 0 else 14
                ncl = 29 if pj == 0 else 28
                prev = None
                for g in range(2):
                    pr = psJ.tile([C0, 15, 29], F32, tag="adj")
                    nc.vector.memset(pr[:, :nr, :ncl], 0.0)
                    for t_, dr, dc in ptaps:
                        for fb in range(4):
                            nc.tensor.matmul(
                                pr[:, dr:dr + GR, dc:dc + OW],
                                lhsT=we0_adj_r[:, fb, t_],
                                rhs=spT_r[:, fb, g * 392:(g + 1) * 392],
                                start=False, stop=False, skip_group_check=True)
                    if dbg and i == 0 and ip == 0 and g == 0:
                        drb = scr.tile([CO, 512], F32, tag="scratch")
                        nc.scalar.copy(drb[:C0, :435], pr[:].rearrange("p a b -> p (a b)"))
                        nc.sync.dma_start(dbg_o["d_recon"][:].rearrange("p a b -> p (a b)"), drb[:C0, :435])
                    if pi == 0:
                        if g == 0:
                            rows = (1, 14)
                        else:
                            cb = scr.tile([CO, 512], F32, tag="scratch")
                            nc.scalar.copy(cb[:C0, :ncl], prev[:, 14, :ncl])
                            nc.vector.tensor_add(pr[:, 0, :ncl], pr[:, 0, :ncl],
                                                 cb[:C0, :ncl])
                            rows = (0, 15)
                    else:
                        rows = (0, 14)
                    c0_, c1_ = (1, ncl) if pj == 0 else (0, ncl)
                    nrp = rows[1] - rows[0]
                    ncp = c1_ - c0_
                    ar0 = g * GR + rows[0]
                    xr0 = 2 * ar0 + pi
                    xc0 = 2 * c0_ + pj
                    d = scr.tile([CO, 512], F32, tag="scratch")
                    dv = d[:C0, :nrp * ncp].rearrange("p (a b) -> p a b", a=nrp)
                    nc.vector.scalar_tensor_tensor(
                        out=dv, in0=pr[:, rows[0]:rows[1], c0_:c1_], scalar=1.0,
                        in1=xp[0:C0, xr0:xr0 + 2 * nrp:2, xc0:xc0 + 2 * ncp:2],
                        op0=OP.mult, op1=OP.subtract)
                    d2 = scr.tile([CO, 512], F32, tag="scratch")
                    nc.scalar.activation(out=d2[:C0, :nrp * ncp], in_=d[:C0, :nrp * ncp],
                                         func=AF.Square,
                                         accum_out=aux0_acc[:, 8 * i + 2 * ip + g:8 * i + 2 * ip + g + 1])
                    prev = pr

        # =============== allreduce 1 ===============
        nc.vector.reduce_sum(sv[:, 0:1], y0_st[:, 0], axis=mybir.AxisListType.X)
        nc.vector.reduce_sum(sv[:, 1:2], y0_st[:, 1], axis=mybir.AxisListType.X)
        nc.vector.reduce_sum(sv[:, 2:3], sc_st[:, 0], axis=mybir.AxisListType.X)
        nc.vector.reduce_sum(sv[:, 3:4], sc_st[:, 1], axis=mybir.AxisListType.X)
        a0c = P0.tile([C0, 1], F32)
        nc.vector.reduce_sum(a0c[:], aux0_acc[:], axis=mybir.AxisListType.X)
        cross_part_sum(a0c[:], C0, 0)
        nc.sync.dma_start(ar1_in[0:128], sv[:, 0:1])
        nc.sync.dma_start(ar1_in[128:256], sv[:, 1:2])
        nc.sync.dma_start(ar1_in[256:384], sv[:, 2:3])
        nc.sync.dma_start(ar1_in[384:512], sv[:, 3:4])
        nc.sync.dma_start(ar1_in[512:513], aux_sb[0:1, 0:1])
        nc.gpsimd.collective_compute(
            "AllReduce", mybir.AluOpType.add, replica_groups=groups,
            ins=[ar1_in[:]], outs=[ar1_out[:]])
        nc.sync.dma_start(sv[:, 0:1], ar1_out[0:128])
        nc.sync.dma_start(sv[:, 1:2], ar1_out[128:256])
        nc.sync.dma_start(sv[:, 2:3], ar1_out[256:384])
        nc.sync.dma_start(sv[:, 3:4], ar1_out[384:512])
        nc.sync.dma_start(aux_sb[0:1, 0:1], ar1_out[512:513])

        s0c, t0c = sv[:, 4:5], sv[:, 5:6]
        bn_coeffs(sv[:, 0:1], sv[:, 1:2], g0_e, b0_e, s0c, t0c)
        ssc, tsc = sv[:, 6:7], sv[:, 7:8]
        bn_coeffs(sv[:, 2:3], sv[:, 3:4], gsc_e, bsc_e, ssc, tsc)
        nc.scalar.mul(aux_sb[0:1, 2:3], aux_sb[0:1, 0:1], 1.0 / s0_denom)

        # =============== build out0 (xpad1) ===============
        nc.vector.memset(xpad1[:, :, 0], 0.0)
        nc.vector.memset(xpad1[:, :, 29], 0.0)
        nc.vector.memset(xpad1[:, :, 1:29, 0:1], 0.0)
        nc.vector.memset(xpad1[:, :, 1:29, 29:30], 0.0)
        nc.scalar.activation(
            out=xpad1[:, :, 1:29, 1:29],
            in_=y0T[:].rearrange("p (n h w) -> p n h w", n=NIMG, h=OH),
            func=AF.Identity, scale=s0c, bias=t0c)
        if dbg:
            nc.sync.dma_start(dbg_o["d_y0T"][:], y0T[:])
            nc.sync.dma_start(dbg_o["d_ysc"][:], yscT[:])
            nc.sync.dma_start(dbg_o["d_out0"][:], xpad1[:])

        ctx1.close()   # free layer-0 weights, y0T, xpool

        # bf16 hi/lo split of out0 for conv1 (feeds topk1: ~17-bit mantissa, flip-safe)
        xp1h = P0.tile([CO, NIMG, 30, 30], mybir.dt.bfloat16)
        xp1l = P0.tile([CO, NIMG, 30, 30], mybir.dt.bfloat16)
        x1f = xpad1[:].rearrange("p a b c -> p (a b c)")
        x1hf = xp1h[:].rearrange("p a b c -> p (a b c)")
        x1lf = xp1l[:].rearrange("p a b c -> p (a b c)")
        for cchunk in range(15):
            slc = slice(cchunk * 480, (cchunk + 1) * 480)
            nc.scalar.copy(x1hf[:, slc], x1f[:, slc])
            rs = scr.tile([CO, 512], F32, tag="scratch")
            nc.vector.tensor_sub(rs[:, :480], x1f[:, slc], x1hf[:, slc])
            nc.scalar.copy(x1lf[:, slc], rs[:, :480])

        # =============== phase 2: layer 1 ===============
        P2 = ctx.enter_context(tc.tile_pool(name="P2", bufs=1))
        BF16 = mybir.dt.bfloat16
        # hi/lo bf16 split of conv1 weights: w = wh + wl + O(2^-17 w)
        we1h = P2.tile([CO, 9, F], BF16)
        we1l = P2.tile([CO, 9, F], BF16)
        w1r = we1_e[:].rearrange("f c a b -> c (a b) f")
        for tap in range(9):
            wf = scr.tile([CO, 512], F32, tag="scratch")
            nc.sync.dma_start(wf[:], w1r[:, tap])
            nc.scalar.copy(we1h[:, tap], wf[:])
            rs = scr.tile([CO, 512], F32, tag="scratch")
            nc.vector.tensor_sub(rs[:], wf[:], we1h[:, tap])
            nc.scalar.copy(we1l[:, tap], rs[:])
        we1_adj = P2.tile([128, 4, CO, 9], F32)
        for fb in range(4):
            nc.sync.dma_start(we1_adj[:, fb],
                              we1_e[fb * 128:(fb + 1) * 128].rearrange("f c a b -> f c (a b)"))
        we1_adj_r = P2.tile([128, 4, 9, CO], F32R)
        nc.scalar.copy(we1_adj_r[:], we1_adj[:].rearrange("p a c t -> p a t c"))
        wp1_sb = P2.tile([CO, F], F32)
        nc.sync.dma_start(wp1_sb[:], wp1_e[:].rearrange("o f a b -> o (f a b)"))
        wp1T_r = P2.tile([128, 4, CO], F32R)
        for fb in range(4):
            pt = psM.tile([CO, 512], F32, tag="mid")
            nc.tensor.transpose(pt[:, :128], wp1_sb[:, fb * 128:(fb + 1) * 128], ident[:])
            nc.scalar.copy(wp1T_r[:, fb], pt[:, :128])
        y1T = P2.tile([CO, PIX], F32)

        for i in range(NIMG):
            spT_r = spTp.tile([128, 4, OH * OW], F32R, tag="spT")
            for s in range(NSTRIP):
                aT = a0p.tile([128, 4, M], F32, tag="aT")
                for fb in range(4):
                    fsl = slice(fb * 128, (fb + 1) * 128)
                    paT = psA.tile([128, M], F32, tag="aT_ps")
                    t = 0
                    for wt_, xt_ in ((we1h, xp1h), (we1h, xp1l), (we1l, xp1h)):
                        for tap, (ki, kj) in enumerate(TAPS):
                            nc.tensor.matmul(
                                paT[:], lhsT=wt_[:, tap, fsl],
                                rhs=xt_[:, i, 4 * s + ki:4 * s + ki + 4, kj:kj + 28],
                                start=(t == 0), stop=(t == 26))
                            t += 1
                    nc.scalar.copy(aT[:, fb], paT[:])
                pa = psA.tile([M, F], F32, tag="a")
                for fb in range(4):
                    nc.tensor.transpose(pa[:, fb * 128:(fb + 1) * 128], aT[:, fb], ident[:])
                a_sb = a0p.tile([M, F], F32, tag="a_sb")
                nc.scalar.copy(a_sb[:], pa[:])
                sp = topk_select(a_sb)
                for fb in range(4):
                    pt = psM.tile([CO, 512], F32, tag="mid")
                    nc.tensor.transpose(pt[:, :M], sp[:, fb * 128:(fb + 1) * 128],
                                        ident[:M, :M])
                    nc.scalar.copy(spT_r[:, fb, s * M:(s + 1) * M], pt[:, :M])

            # proj1 (f32r: output-only) per half-image
            for h in range(2):
                py = psM.tile([CO, 512], F32, tag="mid")
                for fb in range(4):
                    nc.tensor.matmul(
                        py[:, :392], lhsT=wp1T_r[:, fb],
                        rhs=spT_r[:, fb, h * 392:(h + 1) * 392],
                        start=(fb == 0), stop=(fb == 3))
                col = i * 784 + h * 392
                idx = 2 * i + h
                nc.scalar.activation(out=y1T[:, col:col + 392], in_=py[:, :392],
                                     func=AF.Copy, accum_out=y1_st[:, 0, idx:idx + 1])
                sq = scr.tile([CO, 512], F32, tag="scratch")
                nc.scalar.activation(out=sq[:, :392], in_=py[:, :392], func=AF.Square,
                                     accum_out=y1_st[:, 1, idx:idx + 1])

            # adjoint1 (f32r)
            prev = None
            for g in range(2):
                pr = psJ.tile([CO, 16, 30], F32, tag="adj")
                nc.vector.memset(pr[:], 0.0)
                for t, (ki, kj) in enumerate(TAPS):
                    for fb in range(4):
                        nc.tensor.matmul(
                            pr[:, ki:ki + GR, kj:kj + OW],
                            lhsT=we1_adj_r[:, fb, t],
                            rhs=spT_r[:, fb, g * 392:(g + 1) * 392],
                            start=False, stop=False, skip_group_check=True)
                if g == 0:
                    rows = (1, 14)
                else:
                    cb = scr.tile([CO, 512], F32, tag="scratch")
                    nc.scalar.copy(cb[:, :60], prev[:, 14:16].rearrange("p a b -> p (a b)"))
                    nc.vector.tensor_add(pr[:, 0:2], pr[:, 0:2],
                                         cb[:, :60].rearrange("p (a b) -> p a b", a=2))
                    rows = (0, 15)
                nrp = rows[1] - rows[0]
                ar0 = g * GR + rows[0]
                d = scr.tile([CO, 512], F32, tag="scratch")
                dv = d[:, :nrp * 28].rearrange("p (a b) -> p a b", a=nrp)
                nc.vector.scalar_tensor_tensor(
                    out=dv, in0=pr[:, rows[0]:rows[1], 1:29], scalar=1.0,
                    in1=xpad1[:, i, ar0:ar0 + nrp, 1:29],
                    op0=OP.mult, op1=OP.subtract)
                d2 = scr.tile([CO, 512], F32, tag="scratch")
                nc.scalar.activation(out=d2[:, :nrp * 28], in_=d[:, :nrp * 28],
                                     func=AF.Square,
                                     accum_out=aux1_acc[:, 2 * i + g:2 * i + g + 1])
                prev = pr

        # =============== allreduce 2 ===============
        nc.vector.reduce_sum(sv[:, 0:1], y1_st[:, 0], axis=mybir.AxisListType.X)
        nc.vector.reduce_sum(sv[:, 1:2], y1_st[:, 1], axis=mybir.AxisListType.X)
        a1c = P0.tile([CO, 1], F32)
        nc.vector.reduce_sum(a1c[:], aux1_acc[:], axis=mybir.AxisListType.X)
        cross_part_sum(a1c[:], CO, 1)
        nc.sync.dma_start(ar2_in[0:128], sv[:, 0:1])
        nc.sync.dma_start(ar2_in[128:256], sv[:, 1:2])
        nc.sync.dma_start(ar2_in[256:257], aux_sb[0:1, 1:2])
        nc.gpsimd.collective_compute(
            "AllReduce", mybir.AluOpType.add, replica_groups=groups,
            ins=[ar2_in[:]], outs=[ar2_out[:]])
        nc.sync.dma_start(sv[:, 0:1], ar2_out[0:128])
        nc.sync.dma_start(sv[:, 1:2], ar2_out[128:256])
        nc.sync.dma_start(aux_sb[0:1, 1:2], ar2_out[256:257])
        if dbg:
            nc.sync.dma_start(dbg_o["d_y1T"][:], y1T[:])

        s1c, t1c = sv[:, 4:5], sv[:, 5:6]
        bn_coeffs(sv[:, 0:1], sv[:, 1:2], g1_e, b1_e, s1c, t1c)
        nc.scalar.mul(aux_sb[0:1, 3:4], aux_sb[0:1, 1:2], 1.0 / s1_denom)
        tCc = sv[:, 14:15]
        nc.vector.tensor_add(tCc, t1c, tsc)

        # =============== phase 3: out = relu(s1*y1 + ssc*ysc + tC) ===============
        for c in range(PIX // 392):
            i, h = c // 2, c % 2
            sl = slice(392 * c, 392 * (c + 1))
            tmp = scr.tile([CO, 512], F32, tag="scratch")
            nc.scalar.activation(out=tmp[:, :392], in_=yscT[:, sl],
                                 func=AF.Identity, scale=ssc, bias=tCc)
            tmp2 = scr.tile([CO, 512], F32, tag="scratch")
            nc.vector.scalar_tensor_tensor(
                out=tmp2[:, :392], in0=y1T[:, sl], scalar=s1c, in1=tmp[:, :392],
                op0=OP.mult, op1=OP.add)
            tmp3 = scr.tile([CO, 512], F32, tag="scratch")
            nc.scalar.activation(out=tmp3[:, :392], in_=tmp2[:, :392], func=AF.Relu)
            nc.sync.dma_start(
                out_e[i].rearrange("o h w -> o (h w)")[:, 392 * h:392 * (h + 1)],
                tmp3[:, :392])
        nc.sync.dma_start(aux_e[:], aux_sb[0:1, 2:4])

    nc.finalize()
    return nc


_CACHE = {}


def _get_program(n_cores=NCORES, dbg=False):
    key = (n_cores, dbg)
    if key not in _CACHE:
        _CACHE[key] = build(n_cores, dbg)
    return _CACHE[key]


def kernel(**inputs):
    from concourse.bass_utils import run_bass_kernel_spmd
    nc = _get_program()
    x = np.ascontiguousarray(np.asarray(inputs["x"], dtype=np.float32))
    weights = {k: np.ascontiguousarray(np.asarray(v, dtype=np.float32))
               for k, v in inputs.items() if k != "x"}
    in_maps = []
    for c in range(NCORES):
        m = {"x": x[c * NIMG:(c + 1) * NIMG]}
        m.update(weights)
        in_maps.append(m)
    res = run_bass_kernel_spmd(nc, in_maps, list(range(NCORES)))
    out = np.concatenate([res.results[c]["out"].reshape(NIMG, CO, OH, OW)
                          for c in range(NCORES)], axis=0)
    aux = res.results[0]["aux"].reshape(-1)
    return out, np.float32(aux[0]), np.float32(aux[1])
